# revision 37
# baseline (speedup 1.0000x reference)
"""Trainium2 Bass kernel for the 81-step LSTM decoder + masked softmax.

Math (per batch row b):
    z_t = x_t @ W_x + h_{t-1} @ W_h + b          (gates i, f, o, g; 100 each)
    i,f,o = sigmoid(z);  c_t = f*c + i*g;  h_t = o*c_t
    out_t = softmax(where(mask_t, h_t, -inf))

Strategy: data-parallel over batch (4096 -> 8 cores x 512); no collectives.

Device layout is BATCH-major: z is [128b, 400gate] per batch tile (4 tiles of
128, organized as 2 PAIRS sharing a [128, 2, 512] PSUM tile = 2 banks).

Precision plan (rel-err budget 2e-2; ~1.6e-2 in fp64 simulation):
  - xW GEMM: E-chunks {0,1} in bf16, chunks {2,3} as ONE fp8e4 DoubleRow
    matmul (256-row contraction, 2 rows/cycle) -> 3 matmuls/tile vs 4 bf16.
  - everything else bf16 (h, c, y, p, e, em, W_h, hist) except z (PSUM f32),
    s/r (f32) and the final output (f32).

Gate math (sigmoid folded into custom-DVE affine_mul_reduce, one op per
gate product -- the sigma(z) = 0.5*tanh(z/2)+0.5 affine rides the op's
scale/bias):
    y = tanh(0.5 z_ifo)              ACT  (exp_and_others table; no reloads)
    v = (0.5 y_i + 0.5) * g          DVE affine_mul_reduce (g from PSUM)
    u = (0.5 y_f + 0.5) * c          DVE affine_mul_reduce
    c = u + v                        DVE tensor_tensor
    h = (0.5 y_o + 0.5) * c          DVE affine_mul_reduce -> bf16
Tail (4-window pipeline, t-1..t-4 behind): e = exp(h) (ACT); em = e*mask
split -- tiles 2,3 on Pool, tiles 0,1 on DVE; s = row-reduce in halves
(DVE); r = 1/s (DVE reciprocal_approx_fast); ot = em * r (Pool); DMA out.
The DVE tail ops are emitted as FILLERS between dependent gate ops
(u->c and c->h): a dependent DVE op immediately following its producer
pays a ~600ns pipeline-drain penalty (measured: c-ops run 365ns vs
~1000ns bimodal); inserting an independent op in between hides it.
This drain-hiding is worth ~37us total (415us -> 378us).

The PE stream is half-step interleaved (pair A = tiles 01, pair B = 23):
  hwA(t) | xwA(t+1)[0:2] | transpB(t-1) | xwA(t+1)[2:6] | hwB(t) |
  xwB(t+1)[0:3] | transpA(t) | xwB(t+1)[3:6]
so each pair's recurrence chain (tanh -> v/u -> c -> h -> transpose ->
copy -> hW) gets ~0.75 of a step period to close while the PE stays busy.
hist copies: pair A on ACT (after tanhB), pair B on DVE (window head).
Engine queues are pinned (add_dep); chain ops NEVER go on the Pool engine
(its in-order queue + ~2x slower ops stall the recurrence -- measured).
"""

import sys

if "/opt/trn_rl_repo" not in sys.path:
    sys.path.insert(0, "/opt/trn_rl_repo")

import numpy as np

P = 81       # places / timesteps
H = 100      # LSTM units
E = 512      # encoder feature width
B = 4096     # total batch
NCORES = 8
BS = B // NCORES          # 512 batch rows per core
NB = BS // 128            # 4 batch tiles of 128
NHIST = 3                 # recurrent-state ring depth

_PROGRAM = None


def _build_program():
    import concourse.bacc as bacc
    import concourse.mybir as mybir
    from concourse.tile import TileContext
    from concourse.tile_rust import add_dep_helper
    from contextlib import ExitStack

    f32 = mybir.dt.float32
    bf16 = mybir.dt.bfloat16
    fp8 = mybir.dt.float8e4
    TANH = mybir.ActivationFunctionType.Tanh
    EXP = mybir.ActivationFunctionType.Exp
    ADD = mybir.AluOpType.add
    MULT = mybir.AluOpType.mult
    DR = mybir.MatmulPerfMode.DoubleRow

    nc = bacc.Bacc(None, target_bir_lowering=False)

    # ---- DRAM tensors ----
    xbf_d = nc.dram_tensor("xbf", [P, 128, 2, BS], bf16, kind="ExternalInput")
    x8_d = nc.dram_tensor("x8", [P, 128, 2, BS], fp8, kind="ExternalInput")
    wxb_d = nc.dram_tensor("wxb", [128, 2, 400], bf16, kind="ExternalInput")
    wx8_d = nc.dram_tensor("wx8", [128, 2, 400], fp8, kind="ExternalInput")
    C_ID = 0
    C_WHB = 128
    C_MB = C_WHB + 400
    C_TOT = C_MB + P * H
    consts_d = nc.dram_tensor("consts", [128, C_TOT], bf16, kind="ExternalInput")
    h0T_d = nc.dram_tensor("h0T", [H + 1, BS], bf16, kind="ExternalInput")
    out_d = nc.dram_tensor("out", [BS, P, H], f32, kind="ExternalOutput")

    with ExitStack() as ctx:
        tc = ctx.enter_context(TileContext(nc))
        consts = ctx.enter_context(tc.tile_pool(name="consts", bufs=1))
        xpool = ctx.enter_context(tc.tile_pool(name="xpool", bufs=10))
        ypool = ctx.enter_context(tc.tile_pool(name="ypool", bufs=3))
        ppool = ctx.enter_context(tc.tile_pool(name="ppool", bufs=2))
        uvpool = ctx.enter_context(tc.tile_pool(name="uvpool", bufs=2))
        hpool = ctx.enter_context(tc.tile_pool(name="hpool", bufs=5))
        epool = ctx.enter_context(tc.tile_pool(name="epool", bufs=3))
        empool = ctx.enter_context(tc.tile_pool(name="empool", bufs=5))
        otpool = ctx.enter_context(tc.tile_pool(name="otpool", bufs=4))
        spool = ctx.enter_context(tc.tile_pool(name="spool", bufs=6))
        zpool = ctx.enter_context(tc.tile_pool(name="zpool", bufs=3, space="PSUM"))
        htpool = ctx.enter_context(tc.tile_pool(name="htpool", bufs=2, space="PSUM"))

        xtiles = {}

        def fetch_x(t, split=False):
            xb = xpool.tile([128, 2, BS], bf16, name=f"xb_{t}", tag="xb")
            x8 = xpool.tile([128, 2, BS], fp8, name=f"x8_{t}", tag="x8")
            xtiles[t] = (xb, x8)
            if split:
                for c in range(2):
                    nc.sync.dma_start(out=xb[:, c, :], in_=xbf_d[t][:, c, :])
            else:
                nc.sync.dma_start(out=xb, in_=xbf_d[t])
            nc.sync.dma_start(out=x8, in_=x8_d[t])

        # ---- one-time loads ----
        csb = consts.tile([128, C_TOT], bf16)
        nc.sync.dma_start(out=csb[:, 0:C_MB], in_=consts_d[:, 0:C_MB])
        wxb = consts.tile([128, 2, 400], bf16)
        wx8 = consts.tile([128, 2, 400], fp8)
        for c in range(2):
            nc.sync.dma_start(out=wxb[:, c, :], in_=wxb_d[:, c, :])
        nc.sync.dma_start(out=wx8, in_=wx8_d[:, :, :])
        idn = csb[:, C_ID : C_ID + 128]
        whb = csb[0 : H + 1, C_WHB : C_WHB + 400]
        maskb = csb[:, C_MB:C_TOT].rearrange("p (t h) -> p t h", t=P)

        # mask table is only needed from the first softmax tail on
        nc.sync.dma_start(out=csb[:, C_MB:C_TOT], in_=consts_d[:, C_MB:C_TOT])

        # recurrent-state ring h^T [101, 512] bf16; row H = 1.0 (bias rider)
        hist = [consts.tile([H + 1, BS], bf16, name=f"hist{j}") for j in range(NHIST)]
        for j in range(NHIST - 1):
            nc.sync.dma_start(out=hist[j][H : H + 1, :], in_=h0T_d[H : H + 1, :])
        nc.sync.dma_start(out=hist[NHIST - 1], in_=h0T_d[:, :])
        cT = consts.tile([128, NB, H], f32)      # persistent cell state
        nc.vector.memset(cT, 0.0)

        ztiles = {}      # (t, pair) -> [128, 2, 512] psum tile
        htiles = {}      # t -> h tile [128, 4, 100] bf16
        etiles = {}      # t -> (em tile, s tile)
        last_pe = [None]
        last_act = [None]
        last_dve = [None]

        def pin(bi, anchor, reason):
            if anchor is not None:
                add_dep_helper(bi.ins, anchor.ins, sync=False, reason=reason)
            return bi

        def pe(bi):
            pin(bi, last_pe[0], "pe order")
            last_pe[0] = bi

        def act(bi):
            pin(bi, last_act[0], "act order")
            last_act[0] = bi

        def dve(bi):
            pin(bi, last_dve[0], "dve order")
            last_dve[0] = bi

        last_pool = [None]

        def pool(bi):
            pin(bi, last_pool[0], "pool order")
            last_pool[0] = bi

        def xw(t, pair, lo, hi):
            """xW matmuls, mm index range [lo, hi) of this pair's 6
            (3 per tile: bf16 c0, bf16 c1, fp8 DoubleRow c2+c3)."""
            if t >= P:
                return
            key = (t, pair)
            if key not in ztiles:
                ztiles[key] = zpool.tile(
                    [128, 2, 512], f32, name=f"z_{t}_{pair}", tag="z"
                )
            z = ztiles[key]
            xb, x8 = xtiles[t]
            for i in range(lo, hi):
                j, m = divmod(i, 3)
                k = 2 * pair + j
                sl = slice(128 * k, 128 * (k + 1))
                if m < 2:
                    pe(nc.tensor.matmul(
                        z[:, j, 0:400], xb[:, m, sl], wxb[:, m, :],
                        start=(m == 0), stop=False,
                    ))
                else:
                    pe(nc.tensor.matmul(
                        z[:, j, 0:400], x8[:, :, sl], wx8,
                        start=False, stop=False, perf_mode=DR,
                    ))

        def hw(t, pair):
            z = ztiles[(t, pair)]
            for j in range(2):
                k = 2 * pair + j
                pe(nc.tensor.matmul(
                    z[:, j, 0:400],
                    hist[(t - 1) % NHIST][:, 128 * k : 128 * (k + 1)],
                    whb, start=False, stop=True,
                ))

        httiles = {}

        def transp(t, pair):
            """transpose h(t)-pair -> htp psum (PE only; copy emitted later)."""
            htp = htpool.tile([H, 256], bf16, name=f"htp_{t}_{pair}", tag="htp")
            httiles[(t, pair)] = htp
            hq = htiles[t]
            for j in range(2):
                k = 2 * pair + j
                pe(nc.tensor.transpose(
                    htp[:, 128 * j : 128 * (j + 1)], hq[:, k, :], idn
                ))

        def copy_hist(t, pair):
            """PSUM -> SBUF hist copy (pair A on ACT, pair B on DVE)."""
            htp = httiles.pop((t, pair))
            dst = hist[t % NHIST][0:H, 256 * pair : 256 * (pair + 1)]
            if pair == 0:
                act(nc.scalar.copy(dst, htp))
            else:
                dve(nc.vector.tensor_copy(dst, htp))

        def tanh_emit(t, pair):
            z = ztiles[(t, pair)]
            if t not in htiles:
                htiles[t] = hpool.tile([128, NB, H], bf16, name=f"h_{t}", tag="h")
                htiles[("y", t)] = ypool.tile(
                    [128, NB, 300], f32, name=f"y_{t}", tag="y"
                )
                htiles[("uv", t)] = uvpool.tile(
                    [128, 2, NB, H], f32, name=f"uv_{t}", tag="uv"
                )
            y = htiles[("y", t)]
            pr = slice(2 * pair, 2 * pair + 2)
            act(nc.scalar.activation(y[:, pr, :], z[:, :, 0:300], TANH, scale=0.5))

        def dveseg(t, pair, fillers=()):
            z = ztiles[(t, pair)]
            hq = htiles[t]
            y = htiles[("y", t)]
            uv = htiles[("uv", t)]
            pr = slice(2 * pair, 2 * pair + 2)
            jk = spool.tile([128, 4], f32, name=f"jk_{t}_{pair}", tag="jk")
            f1, f2 = (list(fillers) + [None, None])[:2]
            # v = (0.5 y_i + 0.5) * g = sigmoid(z_i) * g  (reads PSUM; frees z)
            dve(nc.vector.affine_mul_reduce(
                uv[:, 1, pr, :], jk[:, 0:1], y[:, pr, 0:100], z[:, :, 300:400],
                0.5, 0.5,
            ))
            # u = sigmoid(z_f) * c
            dve(nc.vector.affine_mul_reduce(
                uv[:, 0, pr, :], jk[:, 1:2], y[:, pr, 100:200], cT[:, pr, :],
                0.5, 0.5,
            ))
            if f1 is not None:
                f1()  # independent tail op hides the u -> c pipeline drain
            # c = u + v
            dve(nc.vector.tensor_tensor(
                cT[:, pr, :], uv[:, 0, pr, :], uv[:, 1, pr, :], op=ADD
            ))
            if f2 is not None:
                f2()  # hides the c -> h drain
            # h = sigmoid(z_o) * c
            dve(nc.vector.affine_mul_reduce(
                hq[:, pr, :], jk[:, 2:3], y[:, pr, 200:300], cT[:, pr, :],
                0.5, 0.5,
            ))

        X = mybir.AxisListType.X

        def make_etiles(t):
            e = epool.tile([128, NB, H], bf16, name=f"e_{t}", tag="e")
            em = empool.tile([128, NB, H], bf16, name=f"em_{t}", tag="em")
            etiles[t] = em
            etiles[("e", t)] = e

        def em_half(t):
            # em tiles 0,1 on DVE -- doubles as a drain-hiding filler
            e = etiles[("e", t)]
            em = etiles[t]
            mk2 = maskb[:, t : t + 1, :].broadcast_to([128, 2, H])
            dve(nc.vector.tensor_tensor(em[:, 0:2, :], e[:, 0:2, :], mk2, op=MULT))

        def tail_exp(t):
            hq = htiles[t]
            e = etiles[("e", t)]
            em = etiles[t]
            act(nc.scalar.activation(e, hq, EXP, scale=1.0))
            mk2 = maskb[:, t : t + 1, :].broadcast_to([128, 2, H])
            pool(nc.gpsimd.tensor_mul(em[:, 2:4, :], e[:, 2:4, :], mk2))

        def red_half(t, half):
            em = etiles[t]
            if half == 0:
                s = spool.tile([128, NB], f32, name=f"s_{t}", tag="s")
                etiles[("s", t)] = s
                dve(nc.vector.tensor_reduce(
                    s[:, 0:2], em[:, 0:2, :], axis=X, op=ADD
                ))
            else:
                s = etiles[("s", t)]
                dve(nc.vector.tensor_reduce(
                    s[:, 2:4], em[:, 2:4, :], axis=X, op=ADD
                ))

        def red_fin(t):
            s = etiles.pop(("s", t))
            em = etiles[t]
            r = spool.tile([128, NB], f32, name=f"r_{t}", tag="r")
            dve(nc.vector.reciprocal_approx_fast(r, s))
            etiles[t] = (em, r)

        def tail_red(t):
            red_half(t, 0)
            red_half(t, 1)
            red_fin(t)

        def tail_ot(t):
            em, r = etiles.pop(t)
            ot = otpool.tile([128, NB, H], f32, name=f"ot_{t}", tag="ot")
            rb = r.rearrange("p (k o) -> p k o", o=1).broadcast_to([128, NB, H])
            pool(nc.gpsimd.tensor_mul(ot, em, rb))
            nc.sync.dma_start(
                out=out_d[:, t, :].rearrange("(k p) h -> p k h", p=128), in_=ot
            )

        # ---- prologue ----
        fetch_x(0, split=True)
        fetch_x(1, split=True)
        fetch_x(2)
        xw(0, 0, 0, 6)
        xw(0, 1, 0, 6)
        hw(0, 0)
        tanh_emit(0, 0)

        # ---- steady-state windows ----
        for t in range(P):
            if t + 3 < P:
                fetch_x(t + 3)
            if t >= 4:
                tail_ot(t - 4)            # Pool ot + DMA out (queue head)
            if t >= 1:
                transp(t - 1, 1)          # PE transpB(t-1) (hB ready)
                copy_hist(t - 1, 1)       # DVE copyB(t-1), queue head
            hw(t, 1)                      # PE: close zB(t) EARLY
            tanh_emit(t, 1)               # ACT tanhB(t)
            fa, fb = [], ()
            if t >= 3:
                tm = t - 3
                fa.append(lambda: red_half(tm, 0))
                fb = (lambda: red_half(tm, 1), lambda: red_fin(tm))
            if t >= 2:
                te = t - 2
                fa.append(lambda: em_half(te))
            if t >= 1:
                make_etiles(t - 1)
            dveseg(t, 0, fa)              # DVE chain A (tanhA ran last window)
            xw(t + 1, 0, 0, 6)            # PE: xwA(t+1)
            if t >= 1:
                tail_exp(t - 1)           # ACT exp (mid) + Pool em
            dveseg(t, 1, fb)              # DVE chain B
            if t + 1 < P:
                transp(t, 0)              # PE transpA(t) (hA ready)
                copy_hist(t, 0)           # ACT copyA(t), after tanhB
            xw(t + 1, 1, 0, 6)            # PE: xwB(t+1)
            if t + 1 < P:
                hw(t + 1, 0)              # PE: close zA(t+1) at window end
                tanh_emit(t + 1, 0)       # ACT tanhA(t+1)
            ztiles.pop((t, 0))
            ztiles.pop((t, 1))
            xtiles.pop(t)

        # ---- epilogue tails ----
        make_etiles(P - 1)
        tail_exp(P - 1)
        em_half(P - 2)
        em_half(P - 1)
        tail_red(P - 3)
        tail_red(P - 2)
        tail_red(P - 1)
        tail_ot(P - 4)
        tail_ot(P - 3)
        tail_ot(P - 2)
        tail_ot(P - 1)

    nc.compile()
    return nc


def _get_program():
    global _PROGRAM
    if _PROGRAM is None:
        _PROGRAM = _build_program()
    return _PROGRAM


def _prep_in_maps(h_enc, h0, W_x, W_h, b, mask):
    import ml_dtypes

    bf16 = ml_dtypes.bfloat16
    f8 = ml_dtypes.float8_e4m3

    h_enc = np.asarray(h_enc, dtype=np.float32)
    h0 = np.asarray(h0, dtype=np.float32)
    W_x = np.asarray(W_x, dtype=np.float32)
    W_h = np.asarray(W_h, dtype=np.float32)
    b = np.asarray(b, dtype=np.float32)
    mask = np.asarray(mask)

    # gate reorder i,f,g,o -> i,f,o,g (no scaling; sigma-form gate math)
    perm = np.concatenate(
        [np.arange(0, 200), np.arange(300, 400), np.arange(200, 300)]
    )
    Wx_dev = W_x[:, perm]                      # [512, 400]
    Wh_dev = W_h[:, perm]
    b_dev = b[perm]

    wxb = np.ascontiguousarray(
        Wx_dev[0:256].reshape(2, 128, 400).transpose(1, 0, 2)
    ).astype(bf16)
    wx8 = np.ascontiguousarray(
        Wx_dev[256:512].reshape(2, 128, 400).transpose(1, 0, 2)
    ).astype(f8)

    C_TOT = 128 + 400 + P * H
    consts = np.zeros((128, C_TOT), np.float32)
    consts[:, 0:128] = np.eye(128, dtype=np.float32)
    consts[0:H, 128:528] = Wh_dev
    consts[H, 128:528] = b_dev
    maskf = np.where(mask, 1.0, 0.0).astype(np.float32).reshape(1, P * H)
    consts[:, 528:] = maskf
    consts = consts.astype(bf16)

    in_maps = []
    xTf = np.empty((P, E, BS), np.float32)
    for c in range(NCORES):
        shard = h_enc[c * BS : (c + 1) * BS]  # [BS, P, E]
        for t in range(P):
            xTf[t] = shard[:, t, :].T
        xbf = np.ascontiguousarray(
            xTf[:, 0:256].reshape(P, 2, 128, BS).transpose(0, 2, 1, 3)
        ).astype(bf16)
        x8 = np.ascontiguousarray(
            xTf[:, 256:512].reshape(P, 2, 128, BS).transpose(0, 2, 1, 3)
        ).astype(f8)
        h0T = np.ascontiguousarray(
            np.concatenate(
                [h0[c * BS : (c + 1) * BS].T, np.ones((1, BS), np.float32)],
                axis=0,
            )
        ).astype(bf16)
        in_maps.append(
            {"xbf": xbf, "x8": x8, "wxb": wxb, "wx8": wx8, "consts": consts,
             "h0T": h0T}
        )
    return in_maps


def run(inputs: dict, trace: bool = False):
    """Run on 8 cores; returns (full_output, exec_time_ns_or_None)."""
    from concourse.bass_utils import run_bass_kernel_spmd

    nc = _get_program()
    in_maps = _prep_in_maps(**inputs)
    res = run_bass_kernel_spmd(
        nc, in_maps, core_ids=list(range(NCORES)), trace=trace
    )
    out = np.concatenate([r["out"] for r in res.results], axis=0)
    return out, res.exec_time_ns


def kernel(**inputs) -> np.ndarray:
    out, _ = run(inputs, trace=False)
    return out
